# revision 4
# baseline (speedup 1.0000x reference)
"""Trainium2 Bass kernel for nn_MultiHeadAttention_37538014167348.

The reference einsum is 'bhqk,bhvd->bhqd' (k and v are independent), so the
attention output factorizes into (sum_k softmax_weights) * (sum_v V). Softmax
rows sum to exactly 1 (also true for the complex softmax), hence:

    out[b, q, :] = (sum_s x[b, s, :]) @ Wv + S * bv     (independent of q)

Q/K/mask/softmax drop out entirely. The kernel computes the row-sum of x, a
complex [1,768]x[768,768] matvec, and broadcasts the resulting row over the
1024 sequence positions.

Sharding over 8 cores: (batch b in 0..3) x (half of the 768 output features),
i.e. data parallel on B and tensor parallel across heads (6+6 of 12 heads).

Complex math is realized in f32: x stays interleaved (re,im) as [S, 2H]; the
weight matvec uses deinterleaved Re/Im planes of Wv; outputs are re/im planes
re-assembled to complex64 on the host.
"""

import os
import sys

import numpy as np

for _p in ("/opt/trn_rl_repo", "/root/.axon_site/_ro/trn_rl_repo"):
    if os.path.isdir(_p) and _p not in sys.path:
        sys.path.append(_p)

from concourse import bacc, mybir
from concourse.tile import TileContext
from concourse.bass_utils import run_bass_kernel_spmd

B, S, H = 4, 1024, 768
HALF = H // 2          # complex output columns per core
NCORES = 8
P = 128                # SBUF partitions
RPP = 2                # x rows packed per partition per tile
TW = 2 * H * RPP       # 3072 f32 per partition per x tile
NT = S // (P * RPP)    # 4 x tiles
KC = H // P            # 6 contraction chunks of 128
F32 = mybir.dt.float32

_NC = None
LAST_RESULTS = None    # stashed BassKernelResults for profiling in test.py


def _build():
    nc = bacc.Bacc(None, target_bir_lowering=False)

    x = nc.dram_tensor("x", [S, 2 * H], F32, kind="ExternalInput")
    cw = nc.dram_tensor("cw", [H, HALF], F32, kind="ExternalInput")   # Re(Wv[:, half])
    dw = nc.dram_tensor("dw", [H, HALF], F32, kind="ExternalInput")   # Im(Wv[:, half])
    brw = nc.dram_tensor("brw", [1, HALF], F32, kind="ExternalInput")  # Re(S*bv[half])
    biw = nc.dram_tensor("biw", [1, HALF], F32, kind="ExternalInput")  # Im(S*bv[half])
    out_re = nc.dram_tensor("out_re", [S, HALF], F32, kind="ExternalOutput")
    out_im = nc.dram_tensor("out_im", [S, HALF], F32, kind="ExternalOutput")

    # x rows s = t*256 + p*2 + r; partition p holds rows (2p, 2p+1) contiguously
    xv = x.rearrange("(t p r) f -> t p (r f)", t=NT, p=P, r=RPP)
    cv = cw.rearrange("(c p) n -> p c n", p=P)
    dv = dw.rearrange("(c p) n -> p c n", p=P)

    with TileContext(nc) as tc:
        with tc.tile_pool(name="sbuf", bufs=1) as pool, \
             tc.tile_pool(name="psum", bufs=1, space="PSUM") as psum, \
             tc.tile_pool(name="dram", bufs=1, space="DRAM") as dram:

            ones = pool.tile([P, 1], F32)
            nc.vector.memset(ones[:], 1.0)

            # ---- stage 1: column-sum x over all 1024 rows -> psum_u [1, 1536]
            # each tile holds rows (2p, 2p+1); both parity halves of the tile
            # accumulate into the same PSUM columns
            NCHUNK = TW // 512  # 6; chunks c and c+3 map to the same columns
            psum_u = psum.tile([1, 2 * H], F32)
            xts = []
            for t in range(NT):
                xt = pool.tile([P, TW], F32, tag=f"x{t}")
                nc.sync.dma_start(out=xt[:], in_=xv[t])
                xts.append(xt)
            for t in range(NT):
                for c in range(NCHUNK):
                    col = (c % 3) * 512
                    nc.tensor.matmul(
                        psum_u[:, col:col + 512],
                        ones[:],
                        xts[t][:, c * 512:(c + 1) * 512],
                        start=(t == 0 and c < 3),
                        stop=(t == NT - 1 and c >= 3),
                    )
            u = pool.tile([1, 2 * H], F32)
            nc.vector.tensor_copy(u[:], psum_u[:])

            # ---- DRAM roundtrip: transpose+deinterleave u into K-major columns
            scratch = dram.tile([1, 2 * H], F32)
            nc.sync.dma_start(out=scratch[:], in_=u[:])
            sv = scratch.rearrange("a (c p t) -> a t p c", c=KC, p=P, t=2)
            a_t = pool.tile([P, KC], F32)   # Re(u) chunks, a_t[p,c] = Re(u[c*128+p])
            b_t = pool.tile([P, KC], F32)   # Im(u) chunks
            nc.sync.dma_start(out=a_t[:], in_=sv[0, 0])
            nc.sync.dma_start(out=b_t[:], in_=sv[0, 1])
            bn_t = pool.tile([P, KC], F32)
            nc.scalar.mul(bn_t[:], b_t[:], -1.0)

            # ---- weights
            c_sb = pool.tile([P, KC, HALF], F32)
            d_sb = pool.tile([P, KC, HALF], F32)
            nc.sync.dma_start(out=c_sb[:], in_=cv)
            nc.sync.dma_start(out=d_sb[:], in_=dv)

            # ---- stage 2: complex matvec u @ Wv_half
            # re = Re(u)@C - Im(u)@D ; im = Re(u)@D + Im(u)@C
            ore = psum.tile([1, HALF], F32)
            oim = psum.tile([1, HALF], F32)
            for c in range(KC):
                nc.tensor.matmul(ore[:], a_t[:, c:c + 1], c_sb[:, c, :],
                                 start=(c == 0), stop=False)
                nc.tensor.matmul(ore[:], bn_t[:, c:c + 1], d_sb[:, c, :],
                                 start=False, stop=(c == KC - 1))
            for c in range(KC):
                nc.tensor.matmul(oim[:], a_t[:, c:c + 1], d_sb[:, c, :],
                                 start=(c == 0), stop=False)
                nc.tensor.matmul(oim[:], b_t[:, c:c + 1], c_sb[:, c, :],
                                 start=False, stop=(c == KC - 1))

            # ---- bias add
            brw_sb = pool.tile([1, HALF], F32)
            biw_sb = pool.tile([1, HALF], F32)
            nc.sync.dma_start(out=brw_sb[:], in_=brw[:])
            nc.sync.dma_start(out=biw_sb[:], in_=biw[:])
            row_re = pool.tile([1, HALF], F32)
            row_im = pool.tile([1, HALF], F32)
            nc.vector.tensor_add(row_re[:], ore[:], brw_sb[:])
            nc.vector.tensor_add(row_im[:], oim[:], biw_sb[:])

            # ---- broadcast row across partitions (PE outer product with ones)
            ones_row = pool.tile([1, P], F32)
            nc.vector.memset(ones_row[:], 1.0)
            pbc_re = psum.tile([P, HALF], F32)
            pbc_im = psum.tile([P, HALF], F32)
            nc.tensor.matmul(pbc_re[:], ones_row[:], row_re[:], start=True, stop=True)
            nc.tensor.matmul(pbc_im[:], ones_row[:], row_im[:], start=True, stop=True)
            bc_re = pool.tile([P, HALF], F32)
            bc_im = pool.tile([P, HALF], F32)
            nc.vector.tensor_copy(bc_re[:], pbc_re[:])
            nc.vector.tensor_copy(bc_im[:], pbc_im[:])
            for rblk in range(S // P):
                nc.sync.dma_start(out=out_re[rblk * P:(rblk + 1) * P, :], in_=bc_re[:])
                nc.sync.dma_start(out=out_im[rblk * P:(rblk + 1) * P, :], in_=bc_im[:])

    nc.finalize()
    return nc


def _get_nc():
    global _NC
    if _NC is None:
        _NC = _build()
    return _NC


def make_in_maps(x, Wv, bv):
    xf = np.ascontiguousarray(x).view(np.float32).reshape(B, S, 2 * H)
    Wv = np.ascontiguousarray(Wv)
    bv = np.ascontiguousarray(bv)
    in_maps = []
    for core in range(NCORES):
        b, j = divmod(core, 2)
        cols = slice(j * HALF, (j + 1) * HALF)
        in_maps.append({
            "x": xf[b],
            "cw": np.ascontiguousarray(Wv[:, cols].real),
            "dw": np.ascontiguousarray(Wv[:, cols].imag),
            "brw": np.ascontiguousarray((np.float32(S) * bv[cols].real))[None, :],
            "biw": np.ascontiguousarray((np.float32(S) * bv[cols].imag))[None, :],
        })
    return in_maps


def kernel(x, Wq, bq, Wk, bk, Wv, bv, mask, trace=False):
    global LAST_RESULTS
    in_maps = make_in_maps(np.asarray(x), np.asarray(Wv), np.asarray(bv))
    res = run_bass_kernel_spmd(_get_nc(), in_maps, core_ids=list(range(NCORES)),
                               trace=trace)
    LAST_RESULTS = res
    out = np.empty((B, S, H), dtype=np.complex64)
    for core in range(NCORES):
        b, j = divmod(core, 2)
        cols = slice(j * HALF, (j + 1) * HALF)
        r = res.results[core]
        out[b, :, cols] = r["out_re"] + 1j * r["out_im"]
    return out


# revision 10
# speedup vs baseline: 1.1743x; 1.1743x over previous
"""Trainium2 Bass kernel for nn_MultiHeadAttention_37538014167348.

The reference einsum is 'bhqk,bhvd->bhqd' (k and v are independent), so the
attention output factorizes into (sum_k softmax_weights) * (sum_v V). Softmax
rows sum to exactly 1 (also true for the complex softmax), hence:

    out[b, q, :] = (sum_s x[b, s, :]) @ Wv + S * bv     (independent of q)

Q/K/mask/softmax drop out entirely. The kernel computes the row-sum of x, a
complex [1,768]x[768,768] matvec, and broadcasts the resulting row over the
1024 sequence positions.

Sharding over 8 cores: (batch b in 0..3) x (half of the 768 output features),
i.e. data parallel on B and tensor parallel across heads (6+6 of 12 heads).

Complex math is realized in f32: x stays interleaved (re,im) as [S, 2H]; the
weight matvec uses deinterleaved Re/Im planes of Wv (host-preshuffled to a
partition-major [128, 6*384] layout so the DMA is fully contiguous); outputs
are re/im planes re-assembled to complex64 on the host.

Per-core dataflow:
  1. x[b] arrives as 4 tiles [128, 3072] (partition p holds rows 2p, 2p+1).
  2. DVE tree-folds the 1024 rows down to deinterleaved tfa/tfb [128, 768]
     (Re/Im of partial column sums; 128 partial rows over s).
  3. 12 stationary matmuls (lhsT = tfa/tfb chunk [128,128], rhs = ones[128,1])
     finish the s-reduction across partitions, leaving u transposed in PSUM
     column form uta/utb [128, 6] -- no DRAM roundtrip transpose needed.
  4. Stage-2 matmuls use a replicated stationary (u column broadcast over all
     128 PE columns) so each accumulation lands PRE-BROADCAST as [128, 384]:
     re = a@C - b@D, im = a@D + b@C, in float32r (1 cycle/row).
  5. Bias rows are DMA-partition-broadcast, added on DVE, replicated 8x along
     free to [128, 3072] so each output plane is one contiguous 1.5MB DMA
     (partition p holds output rows 8p..8p+7).
"""

import os
import sys

import numpy as np

for _p in ("/opt/trn_rl_repo", "/root/.axon_site/_ro/trn_rl_repo"):
    if os.path.isdir(_p) and _p not in sys.path:
        sys.path.append(_p)

from concourse import bacc, mybir
from concourse.tile import TileContext
from concourse.bass_utils import run_bass_kernel_spmd

B, S, H = 4, 1024, 768
HALF = H // 2          # complex output columns per core
NCORES = 8
P = 128                # SBUF partitions
RPP = 2                # x rows packed per partition per tile
TW = 2 * H * RPP       # 3072 f32 per partition per x tile
NT = S // (P * RPP)    # 4 x tiles
KC = H // P            # 6 contraction chunks of 128
QR = S // P            # 8 output rows per partition
F32 = mybir.dt.float32
F32R = mybir.dt.float32r

_NC = None
LAST_RESULTS = None    # stashed BassKernelResults for profiling in test.py


def _build():
    nc = bacc.Bacc(None, target_bir_lowering=False)

    x = nc.dram_tensor("x", [S, 2 * H], F32, kind="ExternalInput")
    # host-preshuffled: cw[p, c*HALF+n] = Re(Wv)[c*128+p, half_cols[n]]
    cw = nc.dram_tensor("cw", [P, KC * HALF], F32, kind="ExternalInput")
    dw = nc.dram_tensor("dw", [P, KC * HALF], F32, kind="ExternalInput")
    brw = nc.dram_tensor("brw", [1, HALF], F32, kind="ExternalInput")  # Re(S*bv)
    biw = nc.dram_tensor("biw", [1, HALF], F32, kind="ExternalInput")  # Im(S*bv)
    out_re = nc.dram_tensor("out_re", [S, HALF], F32, kind="ExternalOutput")
    out_im = nc.dram_tensor("out_im", [S, HALF], F32, kind="ExternalOutput")

    # x rows s = t*256 + p*2 + r; partition p holds rows (2p, 2p+1) contiguously
    xv = x.rearrange("(t p r) f -> t p (r f)", t=NT, p=P, r=RPP)
    # output rows q = p*QR + r so each partition's 8 rows are contiguous 12KB
    ov_re = out_re.rearrange("(p q) n -> p (q n)", p=P, q=QR)
    ov_im = out_im.rearrange("(p q) n -> p (q n)", p=P, q=QR)

    with TileContext(nc) as tc:
        with tc.tile_pool(name="sbuf", bufs=1) as pool, \
             tc.tile_pool(name="psum", bufs=1, space="PSUM") as psum:

            ones = pool.tile([P, 1], F32)
            nc.vector.memset(ones[:], 1.0)

            # ---- weights (contiguous partition-major) + bias broadcast loads
            c_sb = pool.tile([P, KC * HALF], F32)
            d_sb = pool.tile([P, KC * HALF], F32)
            nc.scalar.dma_start(out=c_sb[:], in_=cw[:])
            nc.scalar.dma_start(out=d_sb[:], in_=dw[:])
            # round weights to f32r once (DVE copy performs the rounding)
            c_r = pool.tile([P, KC * HALF], F32R)
            d_r = pool.tile([P, KC * HALF], F32R)
            nc.vector.tensor_copy(c_r[:], c_sb[:])
            nc.vector.tensor_copy(d_r[:], d_sb[:])
            brw_bc = pool.tile([P, HALF], F32)
            biw_bc = pool.tile([P, HALF], F32)
            nc.scalar.dma_start(out=brw_bc[:], in_=brw[:, :].to_broadcast([P, HALF]))
            nc.scalar.dma_start(out=biw_bc[:], in_=biw[:, :].to_broadcast([P, HALF]))

            # ---- stage 1: load x tiles, DVE tree-fold 1024 rows -> 128
            xts = []
            for t in range(NT):
                xt = pool.tile([P, TW], F32, tag=f"x{t}")
                nc.sync.dma_start(out=xt[:], in_=xv[t])
                xts.append(xt)
            t01 = pool.tile([P, TW], F32)
            t23 = pool.tile([P, TW], F32)
            nc.vector.tensor_add(t01[:], xts[0][:], xts[1][:])
            nc.vector.tensor_add(t23[:], xts[2][:], xts[3][:])
            # fold row-parity halves and deinterleave re/im in one pass:
            # view [P, r=2, k=768, t=2]
            v01 = t01.rearrange("p (r k t) -> p r t k", r=RPP, t=2)
            v23 = t23.rearrange("p (r k t) -> p r t k", r=RPP, t=2)
            tfa1 = pool.tile([P, H], F32)
            tfb1 = pool.tile([P, H], F32)
            nc.vector.tensor_add(tfa1[:], v01[:, 0, 0, :], v01[:, 1, 0, :])
            nc.vector.tensor_add(tfb1[:], v01[:, 0, 1, :], v01[:, 1, 1, :])
            tfa = pool.tile([P, H], F32)
            tfb = pool.tile([P, H], F32)
            nc.vector.tensor_add(tfa[:], tfa1[:], v23[:, 0, 0, :])
            nc.vector.tensor_add(tfa[:], tfa[:], v23[:, 1, 0, :])
            nc.vector.tensor_add(tfb[:], tfb1[:], v23[:, 0, 1, :])
            nc.vector.tensor_add(tfb[:], tfb[:], v23[:, 1, 1, :])

            # ---- finish s-reduction across partitions, output in column form:
            # uta[p, c] = Re(u)[c*128+p], utb = Im(u)
            uta = psum.tile([P, KC], F32)
            utb = psum.tile([P, KC], F32)
            for c in range(KC):
                nc.tensor.matmul(uta[:, c:c + 1], tfa[:, c * P:(c + 1) * P],
                                 ones[:], start=True, stop=True)
                nc.tensor.matmul(utb[:, c:c + 1], tfb[:, c * P:(c + 1) * P],
                                 ones[:], start=True, stop=True)

            # ---- stage 2: replicated-stationary matmuls accumulate the
            # complex matvec directly as a [128, 384] broadcast block
            bre = psum.tile([P, HALF], F32)
            bim = psum.tile([P, HALF], F32)
            for c in range(KC):
                rep_a = pool.tile([P, P], F32R, tag="rep_a")
                rep_b = pool.tile([P, P], F32R, tag="rep_b")
                rep_bn = pool.tile([P, P], F32R, tag="rep_bn")
                nc.vector.tensor_copy(rep_a[:], uta[:, c:c + 1].to_broadcast([P, P]))
                nc.vector.tensor_copy(rep_b[:], utb[:, c:c + 1].to_broadcast([P, P]))
                nc.scalar.mul(rep_bn[:], utb[:, c:c + 1].to_broadcast([P, P]), -1.0)
                cc = c_r[:, c * HALF:(c + 1) * HALF]
                dd = d_r[:, c * HALF:(c + 1) * HALF]
                nc.tensor.matmul(bre[:], rep_a[:], cc,
                                 start=(c == 0), stop=False)
                nc.tensor.matmul(bre[:], rep_bn[:], dd,
                                 start=False, stop=(c == KC - 1))
                nc.tensor.matmul(bim[:], rep_a[:], dd,
                                 start=(c == 0), stop=False)
                nc.tensor.matmul(bim[:], rep_b[:], cc,
                                 start=False, stop=(c == KC - 1))

            # ---- bias add + replicate 8x along free for contiguous out DMA
            bc_re = pool.tile([P, HALF], F32)
            bc_im = pool.tile([P, HALF], F32)
            nc.vector.tensor_add(bc_re[:], bre[:], brw_bc[:])
            nc.vector.tensor_add(bc_im[:], bim[:], biw_bc[:])
            bc2_re = pool.tile([P, QR * HALF], F32)
            bc2_im = pool.tile([P, QR * HALF], F32)
            vr = bc_re[:].unsqueeze(1).to_broadcast([P, QR, HALF])
            vi = bc_im[:].unsqueeze(1).to_broadcast([P, QR, HALF])
            nc.vector.tensor_copy(bc2_re.rearrange("p (q n) -> p q n", q=QR), vr)
            nc.vector.tensor_copy(bc2_im.rearrange("p (q n) -> p q n", q=QR), vi)
            nc.sync.dma_start(out=ov_re, in_=bc2_re[:])
            nc.scalar.dma_start(out=ov_im, in_=bc2_im[:])

    nc.finalize()
    return nc


def _get_nc():
    global _NC
    if _NC is None:
        _NC = _build()
    return _NC


def _preshuffle(w_plane, j):
    # [768, 384] half -> [128, 6*384] with row k=c*128+p at (p, c*384..)
    half = w_plane[:, j * HALF:(j + 1) * HALF]           # [768, 384]
    return np.ascontiguousarray(
        half.reshape(KC, P, HALF).transpose(1, 0, 2).reshape(P, KC * HALF))


def make_in_maps(x, Wv, bv):
    xf = np.ascontiguousarray(x).view(np.float32).reshape(B, S, 2 * H)
    Wv = np.ascontiguousarray(Wv)
    bv = np.ascontiguousarray(bv)
    wre, wim = Wv.real.copy(), Wv.imag.copy()
    in_maps = []
    for core in range(NCORES):
        b, j = divmod(core, 2)
        cols = slice(j * HALF, (j + 1) * HALF)
        in_maps.append({
            "x": xf[b],
            "cw": _preshuffle(wre, j),
            "dw": _preshuffle(wim, j),
            "brw": np.ascontiguousarray(np.float32(S) * bv[cols].real)[None, :],
            "biw": np.ascontiguousarray(np.float32(S) * bv[cols].imag)[None, :],
        })
    return in_maps


def kernel(x, Wq, bq, Wk, bk, Wv, bv, mask, trace=False):
    global LAST_RESULTS
    in_maps = make_in_maps(np.asarray(x), np.asarray(Wv), np.asarray(bv))
    res = run_bass_kernel_spmd(_get_nc(), in_maps, core_ids=list(range(NCORES)),
                               trace=trace)
    LAST_RESULTS = res
    out = np.empty((B, S, H), dtype=np.complex64)
    for core in range(NCORES):
        b, j = divmod(core, 2)
        cols = slice(j * HALF, (j + 1) * HALF)
        r = res.results[core]
        out[b, :, cols] = r["out_re"] + 1j * r["out_im"]
    return out


# revision 11
# speedup vs baseline: 1.3836x; 1.1783x over previous
"""Trainium2 Bass kernel for nn_MultiHeadAttention_37538014167348.

The reference einsum is 'bhqk,bhvd->bhqd' (k and v are independent), so the
attention output factorizes into (sum_k softmax_weights) * (sum_v V). Softmax
rows sum to exactly 1 (also true for the complex softmax), hence:

    out[b, q, :] = (sum_s x[b, s, :]) @ Wv + S * bv     (independent of q)

Q/K/mask/softmax drop out entirely. The kernel computes the row-sum of x, a
complex [1,768]x[768,768] matvec, and broadcasts the resulting row over the
1024 sequence positions.

Sharding over 8 cores: (batch b in 0..3) x (half of the 768 output features),
i.e. data parallel on B and tensor parallel across heads (6+6 of 12 heads).

Complex math is realized in f32: x stays interleaved (re,im) as [S, 2H]; the
weight matvec uses deinterleaved Re/Im planes of Wv (host-preshuffled to a
partition-major [128, 6*384] layout so the DMA is fully contiguous); outputs
are re/im planes re-assembled to complex64 on the host.

Per-core dataflow:
  1. x[b] arrives as 4 tiles [128, 3072] (partition p holds rows 2p, 2p+1).
  2. DVE tree-folds the 1024 rows down to deinterleaved tfa/tfb [128, 768]
     (Re/Im of partial column sums; 128 partial rows over s).
  3. 12 stationary matmuls (lhsT = tfa/tfb chunk [128,128], rhs = ones[128,1])
     finish the s-reduction across partitions, leaving u transposed in PSUM
     column form uta/utb [128, 6] -- no DRAM roundtrip transpose needed.
  4. Stage-2 matmuls use a replicated stationary (u column broadcast over all
     128 PE columns) so each accumulation lands PRE-BROADCAST as [128, 384]:
     re = a@C - b@D, im = a@D + b@C, in float32r (1 cycle/row).
  5. Bias rows are DMA-partition-broadcast, added on DVE, replicated 8x along
     free to [128, 3072] so each output plane is one contiguous 1.5MB DMA
     (partition p holds output rows 8p..8p+7).
"""

import os
import sys

import numpy as np

for _p in ("/opt/trn_rl_repo", "/root/.axon_site/_ro/trn_rl_repo"):
    if os.path.isdir(_p) and _p not in sys.path:
        sys.path.append(_p)

from concourse import bacc, mybir
from concourse.tile import TileContext
from concourse.bass_utils import run_bass_kernel_spmd

B, S, H = 4, 1024, 768
HALF = H // 2          # complex output columns per core
NCORES = 8
P = 128                # SBUF partitions
RPP = 2                # x rows packed per partition per tile
TW = 2 * H * RPP       # 3072 f32 per partition per x tile
NT = S // (P * RPP)    # 4 x tiles
KC = H // P            # 6 contraction chunks of 128
QR = S // P            # 8 output rows per partition
F32 = mybir.dt.float32
F32R = mybir.dt.float32r

_NC = None
LAST_RESULTS = None    # stashed BassKernelResults for profiling in test.py


def _build():
    nc = bacc.Bacc(None, target_bir_lowering=False)

    x = nc.dram_tensor("x", [S, 2 * H], F32, kind="ExternalInput")
    # host-preshuffled: cw[p, c*HALF+n] = Re(Wv)[c*128+p, half_cols[n]]
    cw = nc.dram_tensor("cw", [P, KC * HALF], F32, kind="ExternalInput")
    dw = nc.dram_tensor("dw", [P, KC * HALF], F32, kind="ExternalInput")
    brw = nc.dram_tensor("brw", [1, HALF], F32, kind="ExternalInput")  # Re(S*bv)
    biw = nc.dram_tensor("biw", [1, HALF], F32, kind="ExternalInput")  # Im(S*bv)
    out_re = nc.dram_tensor("out_re", [S, HALF], F32, kind="ExternalOutput")
    out_im = nc.dram_tensor("out_im", [S, HALF], F32, kind="ExternalOutput")

    # x rows s = t*256 + p*2 + r; partition p holds rows (2p, 2p+1) contiguously
    xv = x.rearrange("(t p r) f -> t p (r f)", t=NT, p=P, r=RPP)
    # output rows q = p*QR + r so each partition's 8 rows are contiguous 12KB
    ov_re = out_re.rearrange("(p q) n -> p (q n)", p=P, q=QR)
    ov_im = out_im.rearrange("(p q) n -> p (q n)", p=P, q=QR)

    with TileContext(nc) as tc:
        with tc.tile_pool(name="sbuf", bufs=1) as pool, \
             tc.tile_pool(name="psum", bufs=1, space="PSUM") as psum:

            ones = pool.tile([P, 1], F32)
            nc.vector.memset(ones[:], 1.0)

            # ---- weights (contiguous partition-major) + bias broadcast loads
            c_sb = pool.tile([P, KC * HALF], F32)
            d_sb = pool.tile([P, KC * HALF], F32)
            wdma1 = nc.scalar.dma_start(out=c_sb[:], in_=cw[:])
            wdma2 = nc.scalar.dma_start(out=d_sb[:], in_=dw[:])
            # round weights to f32r once (DVE copy performs the rounding)
            c_r = pool.tile([P, KC * HALF], F32R)
            d_r = pool.tile([P, KC * HALF], F32R)
            nc.vector.tensor_copy(c_r[:], c_sb[:])
            nc.vector.tensor_copy(d_r[:], d_sb[:])
            brw_bc = pool.tile([P, HALF], F32)
            biw_bc = pool.tile([P, HALF], F32)
            nc.scalar.dma_start(out=brw_bc[:], in_=brw[:, :].to_broadcast([P, HALF]))
            nc.scalar.dma_start(out=biw_bc[:], in_=biw[:, :].to_broadcast([P, HALF]))

            # ---- stage 1: load x tiles; per-tile strided folds overlap DMA.
            # tile view [P, r=2, k=768, t=2]: fold row parity + deinterleave
            # re/im as each tile lands, then a short tree combine.
            xdmas = []
            pa, pb = [], []
            for t in range(NT):
                xt = pool.tile([P, TW], F32, tag=f"x{t}")
                xdmas.append(nc.sync.dma_start(out=xt[:], in_=xv[t]))
                vt = xt.rearrange("p (r k t) -> p r t k", r=RPP, t=2)
                pa_t = pool.tile([P, H], F32, tag=f"pa{t}")
                pb_t = pool.tile([P, H], F32, tag=f"pb{t}")
                nc.vector.tensor_add(pa_t[:], vt[:, 0, 0, :], vt[:, 1, 0, :])
                nc.vector.tensor_add(pb_t[:], vt[:, 0, 1, :], vt[:, 1, 1, :])
                pa.append(pa_t)
                pb.append(pb_t)
            pa01 = pool.tile([P, H], F32)
            pb01 = pool.tile([P, H], F32)
            nc.vector.tensor_add(pa01[:], pa[0][:], pa[1][:])
            nc.vector.tensor_add(pb01[:], pb[0][:], pb[1][:])
            tfa = pool.tile([P, H], F32)
            tfb = pool.tile([P, H], F32)
            nc.vector.tensor_add(tfa[:], pa01[:], pa[2][:])
            nc.vector.tensor_add(tfa[:], tfa[:], pa[3][:])
            nc.vector.tensor_add(tfb[:], pb01[:], pb[2][:])
            nc.vector.tensor_add(tfb[:], tfb[:], pb[3][:])
            # deliver x before the weights: weights are only needed at stage 2
            from concourse.tile_rust import add_dep_helper
            for w in (wdma1, wdma2):
                add_dep_helper(w.ins, xdmas[-1].ins, sync=False,
                               reason="prioritize x bandwidth over weights")

            # ---- finish s-reduction across partitions, output in column form:
            # uta[p, c] = Re(u)[c*128+p], utb = Im(u)
            uta = psum.tile([P, KC], F32)
            utb = psum.tile([P, KC], F32)
            for c in range(KC):
                nc.tensor.matmul(uta[:, c:c + 1], tfa[:, c * P:(c + 1) * P],
                                 ones[:], start=True, stop=True)
                nc.tensor.matmul(utb[:, c:c + 1], tfb[:, c * P:(c + 1) * P],
                                 ones[:], start=True, stop=True)

            # ---- stage 2: replicated-stationary matmuls accumulate the
            # complex matvec directly as a [128, 384] broadcast block
            bre = psum.tile([P, HALF], F32)
            bim = psum.tile([P, HALF], F32)
            rep_as, rep_bs, rep_bns = [], [], []
            for c in range(KC):
                rep_a = pool.tile([P, P], F32R, tag="rep_a", bufs=KC)
                rep_b = pool.tile([P, P], F32R, tag="rep_b", bufs=KC)
                rep_bn = pool.tile([P, P], F32R, tag="rep_bn", bufs=KC)
                nc.vector.tensor_copy(rep_a[:], uta[:, c:c + 1].to_broadcast([P, P]))
                nc.vector.tensor_copy(rep_b[:], utb[:, c:c + 1].to_broadcast([P, P]))
                nc.scalar.mul(rep_bn[:], utb[:, c:c + 1].to_broadcast([P, P]), -1.0)
                rep_as.append(rep_a)
                rep_bs.append(rep_b)
                rep_bns.append(rep_bn)
            # all re matmuls first so the re output plane can start its DMA
            # while the im plane is still accumulating
            for c in range(KC):
                cc = c_r[:, c * HALF:(c + 1) * HALF]
                dd = d_r[:, c * HALF:(c + 1) * HALF]
                nc.tensor.matmul(bre[:], rep_as[c][:], cc,
                                 start=(c == 0), stop=False)
                nc.tensor.matmul(bre[:], rep_bns[c][:], dd,
                                 start=False, stop=(c == KC - 1))
            for c in range(KC):
                cc = c_r[:, c * HALF:(c + 1) * HALF]
                dd = d_r[:, c * HALF:(c + 1) * HALF]
                nc.tensor.matmul(bim[:], rep_as[c][:], dd,
                                 start=(c == 0), stop=False)
                nc.tensor.matmul(bim[:], rep_bs[c][:], cc,
                                 start=False, stop=(c == KC - 1))

            # ---- bias add + replicate 8x along free for contiguous out DMA
            bc_re = pool.tile([P, HALF], F32)
            bc_im = pool.tile([P, HALF], F32)
            nc.vector.tensor_add(bc_re[:], bre[:], brw_bc[:])
            nc.vector.tensor_add(bc_im[:], bim[:], biw_bc[:])
            vr = bc_re[:].unsqueeze(1).to_broadcast([P, QR, HALF])
            vi = bc_im[:].unsqueeze(1).to_broadcast([P, QR, HALF])
            ovr = ov_re.rearrange("p (q n) -> p q n", q=QR)
            ovi = ov_im.rearrange("p (q n) -> p q n", q=QR)
            nc.sync.dma_start(out=ovr, in_=vr)
            nc.scalar.dma_start(out=ovi, in_=vi)

    nc.finalize()
    return nc


def _get_nc():
    global _NC
    if _NC is None:
        _NC = _build()
    return _NC


def _preshuffle(w_plane, j):
    # [768, 384] half -> [128, 6*384] with row k=c*128+p at (p, c*384..)
    half = w_plane[:, j * HALF:(j + 1) * HALF]           # [768, 384]
    return np.ascontiguousarray(
        half.reshape(KC, P, HALF).transpose(1, 0, 2).reshape(P, KC * HALF))


def make_in_maps(x, Wv, bv):
    xf = np.ascontiguousarray(x).view(np.float32).reshape(B, S, 2 * H)
    Wv = np.ascontiguousarray(Wv)
    bv = np.ascontiguousarray(bv)
    wre, wim = Wv.real.copy(), Wv.imag.copy()
    in_maps = []
    for core in range(NCORES):
        b, j = divmod(core, 2)
        cols = slice(j * HALF, (j + 1) * HALF)
        in_maps.append({
            "x": xf[b],
            "cw": _preshuffle(wre, j),
            "dw": _preshuffle(wim, j),
            "brw": np.ascontiguousarray(np.float32(S) * bv[cols].real)[None, :],
            "biw": np.ascontiguousarray(np.float32(S) * bv[cols].imag)[None, :],
        })
    return in_maps


def kernel(x, Wq, bq, Wk, bk, Wv, bv, mask, trace=False):
    global LAST_RESULTS
    in_maps = make_in_maps(np.asarray(x), np.asarray(Wv), np.asarray(bv))
    res = run_bass_kernel_spmd(_get_nc(), in_maps, core_ids=list(range(NCORES)),
                               trace=trace)
    LAST_RESULTS = res
    out = np.empty((B, S, H), dtype=np.complex64)
    for core in range(NCORES):
        b, j = divmod(core, 2)
        cols = slice(j * HALF, (j + 1) * HALF)
        r = res.results[core]
        out[b, :, cols] = r["out_re"] + 1j * r["out_im"]
    return out
